# revision 10
# baseline (speedup 1.0000x reference)
"""Bahdanau attention layer kernel for Trainium2 (8 NeuronCores, SPMD).

Math (per example b):
  dens1 = h[b] @ W                       [H]
  dens2 = enc[b] @ U                     [T, H]
  pre   = dens1 + dens2 + bias
  tanh_ = tanh(pre @ Wa + ba)            [T, H]
  cij   = tanh_ @ V                      [T]
  alpha = softmax(cij)                   [T]
  out   = alpha @ enc[b]                 [D]

Folding: pre @ Wa = enc @ (U @ Wa) + (h @ W + bias) @ Wa, so with
  UWa = U @ Wa            (host, f64)
  cb  = (h @ W + bias) @ Wa + ba   per example (host, f64)
only ONE [T,D]x[D,H] matmul per example remains on device.

Device layout: enc is loaded transposed (encT: D on partitions) via the
XBAR-transpose DMA (bf16). PE computes M2T = UWa.T @ encT in [H, T]
layout; ACT applies tanh with per-partition bias cb; PE contracts with a
column-replicated V to produce cij broadcast across all 128 partitions;
ACT exponentiates (softmax without max-subtraction: |cij| <= ||V||_1 ~ 18,
exp is safe in fp32) and accumulates chunk sums; DVE reduces esum,
reciprocates, ACT scales e in place by 1/total -> alphas broadcast; DVE
tensor_tensor_reduce(encT * alphas) accumulates the weighted sum.

Sharding: data-parallel over batch, 4 examples per core, weights replicated.
"""

import sys

sys.path.insert(0, "/opt/trn_rl_repo")

import numpy as np
import ml_dtypes

import concourse.bass as bass
import concourse.bacc as bacc
import concourse.mybir as mybir
import concourse.tile as tile
from concourse.bass_utils import run_bass_kernel_spmd

B, T, D, H = 32, 4096, 512, 512
NCORES = 8
EB = B // NCORES  # examples per core
KC = D // 128     # 4 contraction chunks
HC = H // 128     # 4 h chunks
TC = T // 512     # 8 t chunks of 512
NSLAB = 8         # transpose-load slabs per example (T/NSLAB rows each)

F32 = mybir.dt.float32
BF16 = mybir.dt.bfloat16

_CACHE = {}


def build_bass():
    nc = bacc.Bacc(None)

    enc16 = nc.dram_tensor("enc16", [EB, T, D], BF16, kind="ExternalInput")
    uwa = nc.dram_tensor("uwa", [D, H], BF16, kind="ExternalInput")
    vrep = nc.dram_tensor("vrep", [HC, 128, 128], BF16, kind="ExternalInput")
    cbin = nc.dram_tensor("cb", [EB, H], F32, kind="ExternalInput")
    out = nc.dram_tensor("out", [EB, D], F32, kind="ExternalOutput")
    alph = nc.dram_tensor("alphas", [EB, T], F32, kind="ExternalOutput")

    TSLAB = T // NSLAB

    with tile.TileContext(nc) as tc:
        with (
            tc.tile_pool(name="w", bufs=1) as wpool,
            tc.tile_pool(name="encT", bufs=2) as epool,
            tc.tile_pool(name="e", bufs=EB) as ipool,
            tc.tile_pool(name="tanh", bufs=6) as tpool,
            tc.tile_pool(name="junk", bufs=2) as jpool,
            tc.tile_pool(name="small", bufs=4) as spool,
            tc.tile_pool(name="m2", bufs=5, space="PSUM") as m2pool,
            tc.tile_pool(name="cij", bufs=2, space="PSUM") as cpool,
        ):
            # --- replicated weights ---
            uwa_sb = wpool.tile([128, KC, H], BF16)
            nc.sync.dma_start(uwa_sb[:], uwa.rearrange("(c p) h -> p c h", c=KC, p=128))
            vrep_sb = wpool.tile([128, HC, 128], BF16)
            nc.sync.dma_start(vrep_sb[:], vrep.rearrange("c k m -> k c m"))
            cb_raw = wpool.tile([128, EB, HC], F32)
            nc.sync.dma_start(cb_raw[:], cbin.rearrange("e (c p) -> p e c", c=HC, p=128))
            # ACT-owned copy: the sole ACT instruction that waits on the cb DMA,
            # so every tanh below carries only the PE wait (ACT ISA allows 1).
            cb_sb = wpool.tile([128, EB, HC], F32)
            nc.scalar.copy(cb_sb[:], cb_raw[:])
            ws_sb = wpool.tile([128, EB, KC], F32)

            for b in range(EB):
                # --- transposed load: encT[p, k, t] = enc[b, t, 128k+p] ---
                encT = epool.tile([128, KC, T], BF16, tag="encT")
                for s in range(NSLAB):
                    for k in range(KC):
                        nc.sync.dma_start(
                            out=encT[:, k, s * TSLAB : (s + 1) * TSLAB],
                            in_=enc16[b, s * TSLAB : (s + 1) * TSLAB, 128 * k : 128 * (k + 1)],
                            transpose=True,
                        )

                e_t = ipool.tile([128, T], F32, tag="e")
                esum = spool.tile([128, TC], F32, tag="esum")

                for c in range(TC):
                    tsl = slice(c * 512, (c + 1) * 512)
                    # M2T[h, t] = sum_d UWa[d, h] * encT[d, t]
                    ps = []
                    for j in range(HC):
                        p = m2pool.tile([128, 512], F32, tag="m2")
                        for k in range(KC):
                            nc.tensor.matmul(
                                p[:],
                                lhsT=uwa_sb[:, k, 128 * j : 128 * (j + 1)],
                                rhs=encT[:, k, tsl],
                                start=(k == 0),
                                stop=(k == KC - 1),
                            )
                        ps.append(p)
                    cij = cpool.tile([128, 512], F32, tag="cij")
                    for j in range(HC):
                        th = tpool.tile([128, 512], BF16, tag="th")
                        nc.scalar.activation(
                            th[:], ps[j][:], mybir.ActivationFunctionType.Tanh,
                            bias=cb_sb[:, b, j : j + 1],
                        )
                        nc.tensor.matmul(
                            cij[:], lhsT=vrep_sb[:, j, :], rhs=th[:],
                            start=(j == 0), stop=(j == HC - 1),
                        )
                    # e = exp(cij) (every partition holds the same row)
                    nc.scalar.activation(
                        e_t[:, tsl], cij[:], mybir.ActivationFunctionType.Exp,
                        accum_out=esum[:, c : c + 1],
                    )

                tot = spool.tile([128, 1], F32, tag="tot")
                nc.vector.reduce_sum(tot[:], esum[:], axis=mybir.AxisListType.X)
                inv = spool.tile([128, 1], F32, tag="inv")
                nc.vector.reciprocal(inv[:], tot[:])
                # alphas = e / total, in place (broadcast across partitions)
                nc.scalar.mul(e_t[:], e_t[:], inv[:])
                nc.sync.dma_start(alph[b : b + 1, :], e_t[0:1, :])

                # weighted sum: ws[d] = sum_t encT[d, t] * alpha[t]
                acc = spool.tile([128, KC, TC], F32, tag="acc")
                for k in range(KC):
                    junk = jpool.tile([128, 512], F32, tag="junk")
                    for c in range(TC):
                        tsl = slice(c * 512, (c + 1) * 512)
                        nc.vector.scalar_tensor_tensor(
                            out=junk[:],
                            in0=encT[:, k, tsl],
                            scalar=1.0,
                            in1=e_t[:, tsl],
                            op0=mybir.AluOpType.mult,
                            op1=mybir.AluOpType.mult,
                            accum_out=acc[:, k, c : c + 1],
                        )
                    nc.vector.reduce_sum(
                        ws_sb[:, b, k : k + 1], acc[:, k, :], axis=mybir.AxisListType.X
                    )

            nc.sync.dma_start(
                out.rearrange("e (k p) -> p e k", k=KC, p=128), ws_sb[:]
            )

    nc.finalize()
    return nc


def _prep(hidden_state, encoder_outputs, W, U, V, bias, Wa, ba):
    UWa = (U.astype(np.float64) @ Wa.astype(np.float64)).astype(np.float32)
    cb = (
        (hidden_state.astype(np.float64) @ W.astype(np.float64) + bias.astype(np.float64))
        @ Wa.astype(np.float64)
        + ba.astype(np.float64)
    ).astype(np.float32)
    vrep = np.ascontiguousarray(
        np.broadcast_to(
            V.astype(np.float32).reshape(HC, 128, 1), (HC, 128, 128)
        )
    ).astype(ml_dtypes.bfloat16)
    uwa16 = UWa.astype(ml_dtypes.bfloat16)
    enc16 = np.asarray(encoder_outputs).astype(ml_dtypes.bfloat16)
    return enc16, uwa16, vrep, cb


def _install_ntff_shim():
    """Provide antenv.axon_hooks (missing on this image) so trace=True works."""
    try:
        from antenv.axon_hooks import get_axon_ntff_profile_hook  # noqa: F401

        return
    except ImportError:
        pass
    import contextlib
    import ctypes
    import types

    so_path = "/opt/axon/libaxon_pjrt.so"
    lib = ctypes.CDLL(so_path)
    lib.axon_start_nrt_profile.argtypes = [
        ctypes.POINTER(ctypes.c_int64),
        ctypes.c_size_t,
    ]
    lib.axon_start_nrt_profile.restype = ctypes.c_int64
    lib.axon_stop_nrt_profile.argtypes = [ctypes.c_char_p]
    lib.axon_stop_nrt_profile.restype = ctypes.c_int64

    @contextlib.contextmanager
    def _hook(output_dir, device_ids):
        import jax

        jax.devices()
        if device_ids:
            ids = (ctypes.c_int64 * len(device_ids))(*device_ids)
            rc = lib.axon_start_nrt_profile(ids, len(device_ids))
        else:
            rc = lib.axon_start_nrt_profile(None, 0)
        if rc != 0:
            raise RuntimeError(f"axon_start_nrt_profile rc={rc}")
        try:
            yield
        finally:
            n = lib.axon_stop_nrt_profile(str(output_dir).encode())
            print(f"ntff profile: {n} file(s) written to {output_dir}", file=sys.stderr)

    mod = types.ModuleType("antenv.axon_hooks")
    mod.get_axon_ntff_profile_hook = lambda: _hook
    mod.set_axon_ntff_profile_hook = lambda h: None
    sys.modules["antenv.axon_hooks"] = mod


def kernel(hidden_state, encoder_outputs, W, U, V, bias, Wa, ba, _trace=False):
    if _trace:
        _install_ntff_shim()
    hidden_state = np.asarray(hidden_state, dtype=np.float32)
    encoder_outputs = np.asarray(encoder_outputs, dtype=np.float32)
    W = np.asarray(W, dtype=np.float32)
    U = np.asarray(U, dtype=np.float32)
    V = np.asarray(V, dtype=np.float32)
    bias = np.asarray(bias, dtype=np.float32)
    Wa = np.asarray(Wa, dtype=np.float32)
    ba = np.asarray(ba, dtype=np.float32)

    enc16, uwa16, vrep, cb = _prep(
        hidden_state, encoder_outputs, W, U, V, bias, Wa, ba
    )

    if "nc" not in _CACHE:
        _CACHE["nc"] = build_bass()
    nc = _CACHE["nc"]

    in_maps = []
    for core in range(NCORES):
        sl = slice(core * EB, (core + 1) * EB)
        in_maps.append(
            {
                "enc16": np.ascontiguousarray(enc16[sl]),
                "uwa": uwa16,
                "vrep": vrep,
                "cb": np.ascontiguousarray(cb[sl]),
            }
        )

    res = run_bass_kernel_spmd(
        nc, in_maps, core_ids=list(range(NCORES)), trace=_trace
    )
    output = np.concatenate([r["out"] for r in res.results], axis=0)
    alphas = np.concatenate([r["alphas"] for r in res.results], axis=0)
    if _trace:
        return (output, alphas), res
    return output, alphas


# revision 12
# speedup vs baseline: 1.1623x; 1.1623x over previous
"""Bahdanau attention layer kernel for Trainium2 (8 NeuronCores, SPMD).

Math (per example b):
  dens1 = h[b] @ W                       [H]
  dens2 = enc[b] @ U                     [T, H]
  pre   = dens1 + dens2 + bias
  tanh_ = tanh(pre @ Wa + ba)            [T, H]
  cij   = tanh_ @ V                      [T]
  alpha = softmax(cij)                   [T]
  out   = alpha @ enc[b]                 [D]

Folding: pre @ Wa = enc @ (U @ Wa) + (h @ W + bias) @ Wa, so with
  UWa = U @ Wa            (host, f64)
  cb  = (h @ W + bias) @ Wa + ba   per example (host, f64)
only ONE [T,D]x[D,H] matmul per example remains on device.

Device layout: enc is loaded transposed (encT: D on partitions) via the
XBAR-transpose DMA (bf16). PE computes M2T = UWa.T @ encT in [H, T]
layout; ACT applies tanh with per-partition bias cb; PE contracts with a
column-replicated V to produce cij broadcast across all 128 partitions;
ACT exponentiates (softmax without max-subtraction: |cij| <= ||V||_1 ~ 18,
exp is safe in fp32) and accumulates chunk sums; DVE reduces esum,
reciprocates, ACT scales e in place by 1/total -> alphas broadcast; DVE
tensor_tensor_reduce(encT * alphas) accumulates the weighted sum.

Sharding: data-parallel over batch, 4 examples per core, weights replicated.
"""

import sys

sys.path.insert(0, "/opt/trn_rl_repo")

import numpy as np
import ml_dtypes

import concourse.bass as bass
import concourse.bacc as bacc
import concourse.mybir as mybir
import concourse.tile as tile
from concourse.bass_utils import run_bass_kernel_spmd

B, T, D, H = 32, 4096, 512, 512
NCORES = 8
EB = B // NCORES  # examples per core
KC = D // 128     # 4 contraction chunks
HC = H // 128     # 4 h chunks
TC = T // 512     # 8 t chunks of 512
NSLAB = 8         # transpose-load slabs per example (T/NSLAB rows each)

F32 = mybir.dt.float32
BF16 = mybir.dt.bfloat16

_CACHE = {}


def build_bass():
    nc = bacc.Bacc(None)

    enc16 = nc.dram_tensor("enc16", [EB, T, D], BF16, kind="ExternalInput")
    uwa = nc.dram_tensor("uwa", [D, H], BF16, kind="ExternalInput")
    vrep = nc.dram_tensor("vrep", [HC, 128, 128], BF16, kind="ExternalInput")
    cbin = nc.dram_tensor("cb", [EB, H], F32, kind="ExternalInput")
    out = nc.dram_tensor("out", [EB, D], F32, kind="ExternalOutput")
    alph = nc.dram_tensor("alphas", [EB, T], F32, kind="ExternalOutput")

    TSLAB = T // NSLAB

    with tile.TileContext(nc) as tc:
        with (
            tc.tile_pool(name="w", bufs=1) as wpool,
            tc.tile_pool(name="encT", bufs=2) as epool,
            tc.tile_pool(name="e", bufs=2) as ipool,
            tc.tile_pool(name="tanh", bufs=6) as tpool,
            tc.tile_pool(name="junk", bufs=2) as jpool,
            tc.tile_pool(name="small", bufs=4) as spool,
            tc.tile_pool(name="m2", bufs=4, space="PSUM") as m2pool,
            tc.tile_pool(name="cij", bufs=4, space="PSUM") as cpool,
        ):
            # --- replicated weights (scalar HWDGE queue; SP queue is for encT) ---
            uwa_sb = wpool.tile([128, KC, H], BF16)
            nc.scalar.dma_start(uwa_sb[:], uwa.rearrange("(c p) h -> p c h", c=KC, p=128))
            vrep_sb = wpool.tile([128, HC, 128], BF16)
            nc.scalar.dma_start(vrep_sb[:], vrep.rearrange("c k m -> k c m"))
            cb_raw = wpool.tile([128, EB, HC], F32)
            nc.scalar.dma_start(cb_raw[:], cbin.rearrange("e (c p) -> p e c", c=HC, p=128))
            # ACT-owned copy: the sole ACT instruction that waits on the cb DMA,
            # so every tanh below carries only the PE wait (ACT ISA allows 1).
            cb_sb = wpool.tile([128, EB, HC], F32)
            nc.scalar.copy(cb_sb[:], cb_raw[:])
            ws_sb = wpool.tile([128, EB, KC], F32)

            for b in range(EB):
                # --- transposed load: encT[p, k, t] = enc[b, t, 128k+p] ---
                encT = epool.tile([128, KC, T], BF16, tag="encT")
                for k in range(KC):
                    nc.sync.dma_start(
                        out=encT[:, k, :],
                        in_=enc16[b, :, 128 * k : 128 * (k + 1)],
                        transpose=True,
                    )

                e_t = ipool.tile([128, T], F32, tag="e")
                esum = spool.tile([128, TC], F32, tag="esum")
                wacc = spool.tile([128, KC, TC], F32, tag="wacc")

                # t-chunks processed in pairs so consecutive matmuls share the
                # stationary operand (LDWEIGHTS amortization + denser PE).
                for g in range(TC // 2):
                    cs = (2 * g, 2 * g + 1)
                    sl = {c: slice(c * 512, (c + 1) * 512) for c in cs}
                    cij = {c: cpool.tile([128, 512], F32, tag="cij", name=f"cij_{b}_{c}") for c in cs}
                    for j in range(HC):
                        jsl = slice(128 * j, 128 * (j + 1))
                        ps = {c: m2pool.tile([128, 512], F32, tag="m2", name=f"m2_{b}_{j}_{c}") for c in cs}
                        for k in range(KC):
                            for c in cs:
                                nc.tensor.matmul(
                                    ps[c][:],
                                    lhsT=uwa_sb[:, k, jsl],
                                    rhs=encT[:, k, sl[c]],
                                    start=(k == 0),
                                    stop=(k == KC - 1),
                                )
                        for c in cs:
                            th = tpool.tile([128, 512], BF16, tag="th")
                            nc.scalar.activation(
                                th[:], ps[c][:], mybir.ActivationFunctionType.Tanh,
                                bias=cb_sb[:, b, j : j + 1],
                            )
                            nc.tensor.matmul(
                                cij[c][:], lhsT=vrep_sb[:, j, :], rhs=th[:],
                                start=(j == 0), stop=(j == HC - 1),
                            )
                    for c in cs:
                        # e = exp(cij), identical across partitions; esum column
                        # gets the per-chunk total.
                        nc.scalar.activation(
                            e_t[:, sl[c]], cij[c][:], mybir.ActivationFunctionType.Exp,
                            accum_out=esum[:, c : c + 1],
                        )
                        # unnormalized weighted-sum partials chase the exp
                        junk = jpool.tile([128, 512], F32, tag="junk")
                        for k in range(KC):
                            nc.vector.scalar_tensor_tensor(
                                out=junk[:],
                                in0=encT[:, k, sl[c]],
                                scalar=1.0,
                                in1=e_t[:, sl[c]],
                                op0=mybir.AluOpType.mult,
                                op1=mybir.AluOpType.mult,
                                accum_out=wacc[:, k, c : c + 1],
                            )

                tot = spool.tile([128, 1], F32, tag="tot")
                nc.vector.reduce_sum(tot[:], esum[:], axis=mybir.AxisListType.X)
                inv = spool.tile([128, 1], F32, tag="inv")
                nc.vector.reciprocal(inv[:], tot[:])
                # ws = (sum_c wacc) * inv
                wsum = spool.tile([128, KC], F32, tag="wsum")
                nc.vector.reduce_sum(wsum[:], wacc[:], axis=mybir.AxisListType.X)
                nc.vector.tensor_scalar_mul(ws_sb[:, b, :], wsum[:], inv[:])
                # alphas row: scale row 0 in place, then store
                nc.vector.tensor_scalar_mul(e_t[0:1, :], e_t[0:1, :], inv[0:1, :])
                nc.sync.dma_start(alph[b : b + 1, :], e_t[0:1, :])

            nc.sync.dma_start(
                out.rearrange("e (k p) -> p e k", k=KC, p=128), ws_sb[:]
            )

    nc.finalize()
    return nc


def _prep(hidden_state, encoder_outputs, W, U, V, bias, Wa, ba):
    UWa = (U.astype(np.float64) @ Wa.astype(np.float64)).astype(np.float32)
    cb = (
        (hidden_state.astype(np.float64) @ W.astype(np.float64) + bias.astype(np.float64))
        @ Wa.astype(np.float64)
        + ba.astype(np.float64)
    ).astype(np.float32)
    vrep = np.ascontiguousarray(
        np.broadcast_to(
            V.astype(np.float32).reshape(HC, 128, 1), (HC, 128, 128)
        )
    ).astype(ml_dtypes.bfloat16)
    uwa16 = UWa.astype(ml_dtypes.bfloat16)
    enc16 = np.asarray(encoder_outputs).astype(ml_dtypes.bfloat16)
    return enc16, uwa16, vrep, cb


def _install_ntff_shim():
    """Provide antenv.axon_hooks (missing on this image) so trace=True works."""
    try:
        from antenv.axon_hooks import get_axon_ntff_profile_hook  # noqa: F401

        return
    except ImportError:
        pass
    import contextlib
    import ctypes
    import types

    so_path = "/opt/axon/libaxon_pjrt.so"
    lib = ctypes.CDLL(so_path)
    lib.axon_start_nrt_profile.argtypes = [
        ctypes.POINTER(ctypes.c_int64),
        ctypes.c_size_t,
    ]
    lib.axon_start_nrt_profile.restype = ctypes.c_int64
    lib.axon_stop_nrt_profile.argtypes = [ctypes.c_char_p]
    lib.axon_stop_nrt_profile.restype = ctypes.c_int64

    @contextlib.contextmanager
    def _hook(output_dir, device_ids):
        import jax

        jax.devices()
        if device_ids:
            ids = (ctypes.c_int64 * len(device_ids))(*device_ids)
            rc = lib.axon_start_nrt_profile(ids, len(device_ids))
        else:
            rc = lib.axon_start_nrt_profile(None, 0)
        if rc != 0:
            raise RuntimeError(f"axon_start_nrt_profile rc={rc}")
        try:
            yield
        finally:
            n = lib.axon_stop_nrt_profile(str(output_dir).encode())
            print(f"ntff profile: {n} file(s) written to {output_dir}", file=sys.stderr)

    mod = types.ModuleType("antenv.axon_hooks")
    mod.get_axon_ntff_profile_hook = lambda: _hook
    mod.set_axon_ntff_profile_hook = lambda h: None
    sys.modules["antenv.axon_hooks"] = mod


def kernel(hidden_state, encoder_outputs, W, U, V, bias, Wa, ba, _trace=False):
    if _trace:
        _install_ntff_shim()
    hidden_state = np.asarray(hidden_state, dtype=np.float32)
    encoder_outputs = np.asarray(encoder_outputs, dtype=np.float32)
    W = np.asarray(W, dtype=np.float32)
    U = np.asarray(U, dtype=np.float32)
    V = np.asarray(V, dtype=np.float32)
    bias = np.asarray(bias, dtype=np.float32)
    Wa = np.asarray(Wa, dtype=np.float32)
    ba = np.asarray(ba, dtype=np.float32)

    enc16, uwa16, vrep, cb = _prep(
        hidden_state, encoder_outputs, W, U, V, bias, Wa, ba
    )

    if "nc" not in _CACHE:
        _CACHE["nc"] = build_bass()
    nc = _CACHE["nc"]

    in_maps = []
    for core in range(NCORES):
        sl = slice(core * EB, (core + 1) * EB)
        in_maps.append(
            {
                "enc16": np.ascontiguousarray(enc16[sl]),
                "uwa": uwa16,
                "vrep": vrep,
                "cb": np.ascontiguousarray(cb[sl]),
            }
        )

    res = run_bass_kernel_spmd(
        nc, in_maps, core_ids=list(range(NCORES)), trace=_trace
    )
    output = np.concatenate([r["out"] for r in res.results], axis=0)
    alphas = np.concatenate([r["alphas"] for r in res.results], axis=0)
    if _trace:
        return (output, alphas), res
    return output, alphas


# revision 14
# speedup vs baseline: 1.2203x; 1.0500x over previous
"""Bahdanau attention layer kernel for Trainium2 (8 NeuronCores, SPMD).

Math (per example b):
  dens1 = h[b] @ W                       [H]
  dens2 = enc[b] @ U                     [T, H]
  pre   = dens1 + dens2 + bias
  tanh_ = tanh(pre @ Wa + ba)            [T, H]
  cij   = tanh_ @ V                      [T]
  alpha = softmax(cij)                   [T]
  out   = alpha @ enc[b]                 [D]

Folding: pre @ Wa = enc @ (U @ Wa) + (h @ W + bias) @ Wa, so with
  UWa = U @ Wa            (host, f64)
  cb  = (h @ W + bias) @ Wa + ba   per example (host, f64)
only ONE [T,D]x[D,H] matmul per example remains on device.

Device layout: enc is loaded transposed (encT: D on partitions) via the
XBAR-transpose DMA (bf16). PE computes M2T = UWa.T @ encT in [H, T]
layout; ACT applies tanh with per-partition bias cb; PE contracts with a
column-replicated V to produce cij broadcast across all 128 partitions;
ACT exponentiates (softmax without max-subtraction: |cij| <= ||V||_1 ~ 18,
exp is safe in fp32) and accumulates chunk sums; DVE reduces esum,
reciprocates, ACT scales e in place by 1/total -> alphas broadcast; DVE
tensor_tensor_reduce(encT * alphas) accumulates the weighted sum.

Sharding: data-parallel over batch, 4 examples per core, weights replicated.
"""

import sys

sys.path.insert(0, "/opt/trn_rl_repo")

import numpy as np
import ml_dtypes

import concourse.bass as bass
import concourse.bacc as bacc
import concourse.mybir as mybir
import concourse.tile as tile
from concourse.bass_utils import run_bass_kernel_spmd

B, T, D, H = 32, 4096, 512, 512
NCORES = 8
EB = B // NCORES  # examples per core
KC = D // 128     # 4 contraction chunks
HC = H // 128     # 4 h chunks
TC = T // 512     # 8 t chunks of 512
NSLAB = 4         # transpose-load slabs per example (T/NSLAB rows each)

F32 = mybir.dt.float32
BF16 = mybir.dt.bfloat16

_CACHE = {}


def build_bass():
    nc = bacc.Bacc(None)

    enc16 = nc.dram_tensor("enc16", [EB, T, D], BF16, kind="ExternalInput")
    uwa = nc.dram_tensor("uwa", [D, H], BF16, kind="ExternalInput")
    vrep = nc.dram_tensor("vrep", [HC, 128, 128], BF16, kind="ExternalInput")
    cbin = nc.dram_tensor("cb", [EB, H], F32, kind="ExternalInput")
    out = nc.dram_tensor("out", [EB, D], F32, kind="ExternalOutput")
    alph = nc.dram_tensor("alphas", [EB, T], F32, kind="ExternalOutput")

    TSLAB = T // NSLAB

    with tile.TileContext(nc) as tc:
        with (
            tc.tile_pool(name="w", bufs=1) as wpool,
            tc.tile_pool(name="encT", bufs=2) as epool,
            tc.tile_pool(name="e", bufs=2) as ipool,
            tc.tile_pool(name="tanh", bufs=6) as tpool,
            tc.tile_pool(name="junk", bufs=2) as jpool,
            tc.tile_pool(name="small", bufs=4) as spool,
            tc.tile_pool(name="m2", bufs=4, space="PSUM") as m2pool,
            tc.tile_pool(name="cij", bufs=4, space="PSUM") as cpool,
        ):
            # --- replicated weights (scalar HWDGE queue; SP queue is for encT) ---
            uwa_sb = wpool.tile([128, KC, H], BF16)
            nc.scalar.dma_start(uwa_sb[:], uwa.rearrange("(c p) h -> p c h", c=KC, p=128))
            vrep_sb = wpool.tile([128, HC, 128], BF16)
            nc.scalar.dma_start(vrep_sb[:], vrep.rearrange("c k m -> k c m"))
            cb_raw = wpool.tile([128, EB, HC], F32)
            nc.scalar.dma_start(cb_raw[:], cbin.rearrange("e (c p) -> p e c", c=HC, p=128))
            # ACT-owned copy: the sole ACT instruction that waits on the cb DMA,
            # so every tanh below carries only the PE wait (ACT ISA allows 1).
            cb_sb = wpool.tile([128, EB, HC], F32)
            nc.scalar.copy(cb_sb[:], cb_raw[:])
            ws_sb = wpool.tile([128, EB, KC], F32)

            for b in range(EB):
                # --- transposed load: encT[p, k, t] = enc[b, t, 128k+p] ---
                # Slabbed along T (slab-outer) so the first matmuls only wait
                # for the first slab's four transposes, not the whole example.
                encT = epool.tile([128, KC, T], BF16, tag="encT")
                for s in range(NSLAB):
                    ssl = slice(s * TSLAB, (s + 1) * TSLAB)
                    for k in range(KC):
                        nc.sync.dma_start(
                            out=encT[:, k, ssl],
                            in_=enc16[b, ssl, 128 * k : 128 * (k + 1)],
                            transpose=True,
                        )

                e_t = ipool.tile([128, T], F32, tag="e")
                esum = spool.tile([128, TC], F32, tag="esum")
                wacc = spool.tile([128, KC, TC], F32, tag="wacc")

                # t-chunks processed in pairs so consecutive matmuls share the
                # stationary operand (LDWEIGHTS amortization + denser PE).
                for g in range(TC // 2):
                    cs = (2 * g, 2 * g + 1)
                    sl = {c: slice(c * 512, (c + 1) * 512) for c in cs}
                    cij = {c: cpool.tile([128, 512], F32, tag="cij", name=f"cij_{b}_{c}") for c in cs}
                    for j in range(HC):
                        jsl = slice(128 * j, 128 * (j + 1))
                        ps = {c: m2pool.tile([128, 512], F32, tag="m2", name=f"m2_{b}_{j}_{c}") for c in cs}
                        for k in range(KC):
                            for c in cs:
                                nc.tensor.matmul(
                                    ps[c][:],
                                    lhsT=uwa_sb[:, k, jsl],
                                    rhs=encT[:, k, sl[c]],
                                    start=(k == 0),
                                    stop=(k == KC - 1),
                                )
                        for c in cs:
                            th = tpool.tile([128, 512], BF16, tag="th")
                            nc.scalar.activation(
                                th[:], ps[c][:], mybir.ActivationFunctionType.Tanh,
                                bias=cb_sb[:, b, j : j + 1],
                            )
                            nc.tensor.matmul(
                                cij[c][:], lhsT=vrep_sb[:, j, :], rhs=th[:],
                                start=(j == 0), stop=(j == HC - 1),
                            )
                    for c in cs:
                        # e = exp(cij), identical across partitions; esum column
                        # gets the per-chunk total.
                        nc.scalar.activation(
                            e_t[:, sl[c]], cij[c][:], mybir.ActivationFunctionType.Exp,
                            accum_out=esum[:, c : c + 1],
                        )
                        # unnormalized weighted-sum partials chase the exp
                        junk = jpool.tile([128, 512], F32, tag="junk")
                        for k in range(KC):
                            nc.vector.scalar_tensor_tensor(
                                out=junk[:],
                                in0=encT[:, k, sl[c]],
                                scalar=1.0,
                                in1=e_t[:, sl[c]],
                                op0=mybir.AluOpType.mult,
                                op1=mybir.AluOpType.mult,
                                accum_out=wacc[:, k, c : c + 1],
                            )

                tot = spool.tile([128, 1], F32, tag="tot")
                nc.vector.reduce_sum(tot[:], esum[:], axis=mybir.AxisListType.X)
                inv = spool.tile([128, 1], F32, tag="inv")
                nc.vector.reciprocal(inv[:], tot[:])
                # ws = (sum_c wacc) * inv
                wsum = spool.tile([128, KC], F32, tag="wsum")
                nc.vector.reduce_sum(wsum[:], wacc[:], axis=mybir.AxisListType.X)
                nc.vector.tensor_scalar_mul(ws_sb[:, b, :], wsum[:], inv[:])
                # alphas row: scale row 0 in place, then store
                nc.vector.tensor_scalar_mul(e_t[0:1, :], e_t[0:1, :], inv[0:1, :])
                nc.sync.dma_start(alph[b : b + 1, :], e_t[0:1, :])

            nc.sync.dma_start(
                out.rearrange("e (k p) -> p e k", k=KC, p=128), ws_sb[:]
            )

    nc.finalize()
    return nc


def _prep(hidden_state, encoder_outputs, W, U, V, bias, Wa, ba):
    UWa = (U.astype(np.float64) @ Wa.astype(np.float64)).astype(np.float32)
    cb = (
        (hidden_state.astype(np.float64) @ W.astype(np.float64) + bias.astype(np.float64))
        @ Wa.astype(np.float64)
        + ba.astype(np.float64)
    ).astype(np.float32)
    vrep = np.ascontiguousarray(
        np.broadcast_to(
            V.astype(np.float32).reshape(HC, 128, 1), (HC, 128, 128)
        )
    ).astype(ml_dtypes.bfloat16)
    uwa16 = UWa.astype(ml_dtypes.bfloat16)
    enc16 = np.asarray(encoder_outputs).astype(ml_dtypes.bfloat16)
    return enc16, uwa16, vrep, cb


def _install_ntff_shim():
    """Provide antenv.axon_hooks (missing on this image) so trace=True works."""
    try:
        from antenv.axon_hooks import get_axon_ntff_profile_hook  # noqa: F401

        return
    except ImportError:
        pass
    import contextlib
    import ctypes
    import types

    so_path = "/opt/axon/libaxon_pjrt.so"
    lib = ctypes.CDLL(so_path)
    lib.axon_start_nrt_profile.argtypes = [
        ctypes.POINTER(ctypes.c_int64),
        ctypes.c_size_t,
    ]
    lib.axon_start_nrt_profile.restype = ctypes.c_int64
    lib.axon_stop_nrt_profile.argtypes = [ctypes.c_char_p]
    lib.axon_stop_nrt_profile.restype = ctypes.c_int64

    @contextlib.contextmanager
    def _hook(output_dir, device_ids):
        import jax

        jax.devices()
        if device_ids:
            ids = (ctypes.c_int64 * len(device_ids))(*device_ids)
            rc = lib.axon_start_nrt_profile(ids, len(device_ids))
        else:
            rc = lib.axon_start_nrt_profile(None, 0)
        if rc != 0:
            raise RuntimeError(f"axon_start_nrt_profile rc={rc}")
        try:
            yield
        finally:
            n = lib.axon_stop_nrt_profile(str(output_dir).encode())
            print(f"ntff profile: {n} file(s) written to {output_dir}", file=sys.stderr)

    mod = types.ModuleType("antenv.axon_hooks")
    mod.get_axon_ntff_profile_hook = lambda: _hook
    mod.set_axon_ntff_profile_hook = lambda h: None
    sys.modules["antenv.axon_hooks"] = mod


def kernel(hidden_state, encoder_outputs, W, U, V, bias, Wa, ba, _trace=False):
    if _trace:
        _install_ntff_shim()
    hidden_state = np.asarray(hidden_state, dtype=np.float32)
    encoder_outputs = np.asarray(encoder_outputs, dtype=np.float32)
    W = np.asarray(W, dtype=np.float32)
    U = np.asarray(U, dtype=np.float32)
    V = np.asarray(V, dtype=np.float32)
    bias = np.asarray(bias, dtype=np.float32)
    Wa = np.asarray(Wa, dtype=np.float32)
    ba = np.asarray(ba, dtype=np.float32)

    enc16, uwa16, vrep, cb = _prep(
        hidden_state, encoder_outputs, W, U, V, bias, Wa, ba
    )

    if "nc" not in _CACHE:
        _CACHE["nc"] = build_bass()
    nc = _CACHE["nc"]

    in_maps = []
    for core in range(NCORES):
        sl = slice(core * EB, (core + 1) * EB)
        in_maps.append(
            {
                "enc16": np.ascontiguousarray(enc16[sl]),
                "uwa": uwa16,
                "vrep": vrep,
                "cb": np.ascontiguousarray(cb[sl]),
            }
        )

    res = run_bass_kernel_spmd(
        nc, in_maps, core_ids=list(range(NCORES)), trace=_trace
    )
    output = np.concatenate([r["out"] for r in res.results], axis=0)
    alphas = np.concatenate([r["alphas"] for r in res.results], axis=0)
    if _trace:
        return (output, alphas), res
    return output, alphas
